# revision 25
# baseline (speedup 1.0000x reference)
"""GATv2 attention scores kernel for Trainium2 (8 NeuronCores, Bass/Tile).

Computes attn = softmax_j( sum_d a[h,d] * silu(q[b,h,i,d] + k[b,h,j,d]) )
for q,k: [B,H,N,D] = [16,8,256,32], output [B,H,N,N] f32.

Sharding: the 128 (b,h) pairs are data-parallel; each of the 8 cores
handles 16 pairs. No collectives.

Per-core algorithm (per pair):
  - Host packs k^T replicated 4x over partitions: kT4[32m+d, j] = k[j,d]
    (fp16), q packed as per-group columns qT4[32m+d, g] = q[4g+m, d] (f32),
    and a block-diagonal reduction matrix ablk[32m+d, m'] = (m==m')*a[h,d].
  - DVE tensor_scalar_add (fp16, 2x mode): S[32m+d, g*256+j] = q[4g+m,d]+k[j,d]
  - ScalarE Silu in big batches.
  - TensorE matmul with ablk (zero-padded to M=32) reduces over d: each
    matmul emits scores for 8 query rows, scattered at PSUM partition
    bases {0,32,64,96}.
  - VectorE copies exit PSUM strips to SBUF (fp16); 16 small TensorE
    "gather" matmuls with two constant 0/1 permutation matrices densify
    the scattered rows into one dense [128, 512] PSUM tile per pair.
  - ScalarE Exp with accum_out gives exp + row sums; VectorE reciprocal +
    tensor_scalar_mul normalizes; DMA out. All Exp ops are ordered after
    the Silu stream via nosync dep edges so the ACT table set switches
    only twice per kernel.

mask is all-False for this problem (spec fill=zeros): if a nonzero mask
is ever passed, an exact host-side renormalization fallback is applied.
scale is unused by the module.
"""

import os
import numpy as np
from contextlib import ExitStack

import concourse.bass as bass
import concourse.bacc as bacc
import concourse.mybir as mybir
import concourse.tile as tile
import bass_rust as _bass_rust
from concourse.bass_utils import run_bass_kernel_spmd

B, H, N, D = 16, 8, 256, 32
NCORES = 8
PAIRS = (B * H) // NCORES      # 16 (b,h) pairs per core
G = N // 4                     # 64 groups of 4 query rows
CHUNK_G = int(os.environ.get("GAT_CHUNK_G", "32"))   # groups per silu batch
NCHUNKS = G // CHUNK_G
NBANKS = 8                     # PSUM banks used per pair
ACT_COPY_EIGHTHS = int(os.environ.get("GAT_ACT_COPY_EIGHTHS", "0"))
SADD_BUFS = int(os.environ.get("GAT_SADD_BUFS", "4"))
SSIL_BUFS = int(os.environ.get("GAT_SSIL_BUFS", "4"))
EPOOL_BUFS = int(os.environ.get("GAT_EPOOL_BUFS", "3"))
PSTRIP_BUFS = int(os.environ.get("GAT_PSTRIP_BUFS", "3"))
DEFER = int(os.environ.get("GAT_DEFER", "16"))
ACT_DMA_PAIRS = int(os.environ.get("GAT_ACT_DMA_PAIRS", "0"))        # pairs per softmax block

FP16 = mybir.dt.float16
FP32 = mybir.dt.float32

_cache = {}


def build_program() -> bacc.Bacc:
    if "nc" in _cache:
        return _cache["nc"]
    nc = bacc.Bacc("TRN2")
    kT4_d = nc.declare_dram_parameter("kT4", [PAIRS, 128, N], FP16, isOutput=False)
    qT4_d = nc.declare_dram_parameter("qT4", [PAIRS, 128, G], FP32, isOutput=False)
    ablk_d = nc.declare_dram_parameter("ablk", [PAIRS, 128, 32], FP16, isOutput=False)
    perm_d = nc.declare_dram_parameter("perm", [128, 64], FP16, isOutput=False)
    out_d = nc.declare_dram_parameter("out", [PAIRS, N, N], FP32, isOutput=True)

    with ExitStack() as ctx:
        tc = ctx.enter_context(tile.TileContext(nc))
        inp = ctx.enter_context(tc.tile_pool(name="inp", bufs=3))
        cpool = ctx.enter_context(tc.tile_pool(name="cpool", bufs=1))
        sadd = ctx.enter_context(tc.tile_pool(name="sadd", bufs=SADD_BUFS))
        ssil = ctx.enter_context(tc.tile_pool(name="ssil", bufs=SSIL_BUFS))
        # strip psum tiles: 2 banks each, 3 in flight = 6 banks
        pstrip = ctx.enter_context(tc.tile_pool(name="pstrip", bufs=PSTRIP_BUFS, space="PSUM"))
        # dense psum tile: 1 bank, 2 in flight
        pdense = ctx.enter_context(tc.tile_pool(name="pdense", bufs=2, space="PSUM"))
        epool = ctx.enter_context(tc.tile_pool(name="epool", bufs=EPOOL_BUFS))
        dpool = ctx.enter_context(tc.tile_pool(name="dpool", bufs=DEFER + 2))
        xpool = ctx.enter_context(tc.tile_pool(name="xpool", bufs=3))
        rpool = ctx.enter_context(tc.tile_pool(name="rpool", bufs=3))
        spool = ctx.enter_context(tc.tile_pool(name="spool", bufs=DEFER + 2))

        exit_ctr = 0
        silu_insts = []
        # resident constants: permutation matrices for the gather matmuls
        pm = cpool.tile([128, 64], FP16, name="pm", tag="pm")
        nc.sync.dma_start(pm[:], perm_d[:])

        def phase1(p):
            kt = inp.tile([128, N], FP16, tag="kt")
            nc.sync.dma_start(kt[:], kT4_d[p])
            qt = inp.tile([128, G], FP32, tag="qt")
            nc.sync.dma_start(qt[:], qT4_d[p])
            ab = inp.tile([128, 32], FP16, tag="ab")
            nc.sync.dma_start(ab[:], ablk_d[p])

            Dn = dpool.tile([128, 2 * N], FP32, tag="dense")
            P2 = pdense.tile([128, 512], FP32, name="p2", tag="p2")
            pbanks = {}
            gq = 0  # running strip counter t
            nonlocal exit_ctr
            if p == 0:
                # fine-grained opening chunks: ScalarE starts sooner
                plan = [4, 4, 8]
            else:
                plan = []
            rem = G - sum(plan)
            while rem > 0:
                take = min(CHUNK_G, rem)
                plan.append(take)
                rem -= take
            assert sum(plan) == G
            g0 = 0
            for csz in plan:
                Sa = sadd.tile([128, CHUNK_G * N], FP16, tag="sa")
                for gl in range(csz):
                    g = g0 + gl
                    nc.vector.tensor_scalar_add(
                        Sa[:, gl * N:(gl + 1) * N], kt[:], qt[:, g:g + 1]
                    )
                Ss = ssil.tile([128, CHUNK_G * N], FP16, tag="ss")
                si = nc.scalar.activation(
                    Ss[:, :csz * N], Sa[:, :csz * N],
                    mybir.ActivationFunctionType.Silu
                )
                silu_insts.append(si)
                g0 += csz
                # strip matmuls: strip t covers query rows i = 8t + 4*g2 + m
                # at psum tile pi = t>>3, partition 32*(t&3)+m,
                # free 512*((t>>2)&1) + 256*g2 + j
                for tl in range(csz // 2):
                    t = gq
                    gq += 1
                    pi, sg, c_ = t >> 3, (t >> 2) & 1, t & 3
                    if (t & 7) == 0:
                        pbanks[pi] = pstrip.tile(
                            [128, 1024], FP32, name="pbank", tag="pbank"
                        )
                    # M=32 with zero-padded lhsT: rows 4..31 of each
                    # 32-block are written as zeros (keeps PSUM NaN-free
                    # for the gather matmuls)
                    nc.tensor.matmul(
                        pbanks[pi][32 * c_:32 * c_ + 32, 512 * sg:512 * sg + 512],
                        ab[:, :],
                        Ss[:, tl * 512:(tl + 1) * 512],
                        start=True, stop=True,
                        tile_position=(0, 32 * c_),
                    )
                    if (t & 7) == 7:
                        # tile complete: exit PSUM -> SBUF
                        Eb = epool.tile([128, 1024], FP16, tag="eb")
                        exit_ctr += 1
                        if (exit_ctr % 8) < ACT_COPY_EIGHTHS:
                            nc.scalar.copy(Eb[:], pbanks[pi][:, :])
                        else:
                            nc.vector.tensor_copy(Eb[:], pbanks[pi][:, :])
                        # gather matmuls: densify 64 rows of this tile into P2
                        # kappa = 2*sigma + g2 selects a 256-col block of Eb;
                        # target rows 64*(pi&1)+32*sigma + (8c+4*g2+m)
                        for kap in range(4):
                            sg2, g2 = kap >> 1, kap & 1
                            row0 = 64 * (pi & 1) + 32 * sg2
                            nc.tensor.matmul(
                                P2[row0:row0 + 32,
                                   256 * (pi >> 1):256 * (pi >> 1) + 256],
                                pm[:, 32 * g2:32 * g2 + 32],
                                Eb[:, 256 * kap:256 * kap + 256],
                                start=(g2 == 0), stop=(g2 == 1),
                                tile_position=(0, row0),
                                skip_group_check=True,
                            )
            # dense exit: P2 -> Dn (bitcast fp16 for DVE 2x byte copy)
            nc.vector.tensor_copy(Dn[:].bitcast(FP16), P2[:, :].bitcast(FP16))
            return Dn

        def phase2(p, Dn):
            X = xpool.tile([128, 2 * N], FP32, tag="x")
            sm = spool.tile([128, 4], FP32, tag="sm")
            for h2 in range(2):
                ei = nc.scalar.activation(
                    X[:, h2 * N:(h2 + 1) * N],
                    Dn[:, h2 * N:(h2 + 1) * N],
                    mybir.ActivationFunctionType.Exp,
                    accum_out=sm[:, h2:h2 + 1],
                )
                if exp_gate is not None:
                    # ordering-only edge: batch Exp ops after a chosen Silu
                    # in the static ACT stream (few table switches)
                    _bass_rust.add_dep_helper(
                        ei.ins, exp_gate.ins, sync=False,
                        reason="batch exp after silu (act table)",
                    )
            nc.vector.reciprocal(sm[:, 2:4], sm[:, 0:2])
            R = rpool.tile([128, 2 * N], FP32, tag="r")
            for h2 in range(2):
                nc.vector.tensor_scalar_mul(
                    R[:, h2 * N:(h2 + 1) * N],
                    X[:, h2 * N:(h2 + 1) * N],
                    sm[:, 2 + h2:3 + h2],
                )
            # late pairs issue their output DMA from the (by then idle)
            # ScalarE HWDGE queue to relieve the SP issue backlog at the tail
            eng = nc.scalar if p >= PAIRS - ACT_DMA_PAIRS else nc.sync
            for h2 in range(2):
                eng.dma_start(
                    out_d[p, 128 * h2:128 * (h2 + 1), :],
                    R[:, h2 * N:(h2 + 1) * N],
                )

        # phase1 for all pairs; exp batches released at two points to
        # shorten the end-of-kernel tail while keeping table switches rare
        dns = []
        split = int(os.environ.get("GAT_EXP_SPLIT", str(PAIRS - 4)))
        gate_idx = {}
        for p in range(PAIRS):
            dns.append((p, phase1(p)))
            gate_idx[p] = len(silu_insts) - 1
        exp_gate = None
        first_half = [i for i in range(PAIRS) if i < split]
        second_half = [i for i in range(PAIRS) if i >= split]
        if first_half:
            exp_gate = silu_insts[gate_idx[max(first_half)]]
            for p in first_half:
                phase2(p, dns[p][1])
        exp_gate = silu_insts[-1]
        for p in second_half:
            phase2(p, dns[p][1])

    nc.compile()
    _cache["nc"] = nc
    return nc


def prepare_in_maps(q, k, attention):
    q = np.asarray(q, dtype=np.float32)
    k = np.asarray(k, dtype=np.float32)
    a = np.asarray(attention, dtype=np.float32).reshape(H, D)
    BH = B * H
    qf = q.reshape(BH, N, D)
    kf = k.reshape(BH, N, D)
    # kT4[p, 32m+d, j] = k[p, j, d]
    kT4 = np.tile(kf.transpose(0, 2, 1), (1, 4, 1)).astype(np.float16)
    # qT4[p, 32m+d, g] = q[p, 4g+m, d]
    qT4 = (
        qf.reshape(BH, G, 4, D)
        .transpose(0, 2, 3, 1)
        .reshape(BH, 128, G)
        .astype(np.float32)
    )
    # ablk[p, 32m+d, m'] = (m==m') * a[h(p), d], zero-padded to 32 cols
    ab = np.zeros((BH, 128, 32), np.float16)
    hh = np.arange(BH) % H
    a16 = a.astype(np.float16)
    for m in range(4):
        ab[:, 32 * m:32 * (m + 1), m] = a16[hh]
    # permutation matrices for the gather matmuls:
    # perm[32c+m, 32*g2 + (8c+4g2+m)] = 1
    perm = np.zeros((128, 64), np.float16)
    for c in range(4):
        for g2 in range(2):
            for m in range(4):
                perm[32 * c + m, 32 * g2 + 8 * c + 4 * g2 + m] = 1.0
    in_maps = []
    for c in range(NCORES):
        s = slice(c * PAIRS, (c + 1) * PAIRS)
        in_maps.append(
            {
                "kT4": np.ascontiguousarray(kT4[s]),
                "qT4": np.ascontiguousarray(qT4[s]),
                "ablk": np.ascontiguousarray(ab[s]),
                "perm": perm,
            }
        )
    return in_maps


def unshard_output(results) -> np.ndarray:
    outs = [np.asarray(r["out"]) for r in results]
    return np.concatenate(outs, axis=0).reshape(B, H, N, N).astype(np.float32)


def kernel(q, k, scale, mask, attention) -> np.ndarray:
    nc = build_program()
    in_maps = prepare_in_maps(q, k, attention)
    res = run_bass_kernel_spmd(nc, in_maps, list(range(NCORES)))
    attn = unshard_output(res.results)
    mask = np.asarray(mask)
    if mask.any():
        # exact post-hoc masking: softmax with -inf masked scores equals
        # zeroing masked probabilities and renormalizing
        keep = ~np.broadcast_to(mask, attn.shape)
        kept = attn * keep
        denom = kept.sum(-1, keepdims=True)
        nkeep = keep.sum(-1, keepdims=True)
        uniform = np.where(nkeep > 0, keep / np.maximum(nkeep, 1), 1.0 / N)
        attn = np.where(denom > 0, kept / np.maximum(denom, 1e-38), uniform)
        attn = attn.astype(np.float32)
    return attn



# revision 28
# speedup vs baseline: 1.0037x; 1.0037x over previous
"""GATv2 attention scores kernel for Trainium2 (8 NeuronCores, Bass/Tile).

Computes attn = softmax_j( sum_d a[h,d] * silu(q[b,h,i,d] + k[b,h,j,d]) )
for q,k: [B,H,N,D] = [16,8,256,32], output [B,H,N,N] f32.

Sharding: the 128 (b,h) pairs are data-parallel; each of the 8 cores
handles 16 pairs. No collectives.

Per-core algorithm (per pair):
  - Host packs k^T replicated 4x over partitions: kT4[32m+d, j] = k[j,d]
    (fp16), q packed as per-group columns qT4[32m+d, g] = q[4g+m, d] (f32),
    and a block-diagonal reduction matrix ablk[32m+d, m'] = (m==m')*a[h,d].
  - DVE tensor_scalar_add (fp16, 2x mode): S[32m+d, g*256+j] = q[4g+m,d]+k[j,d]
  - ScalarE Silu in big batches.
  - TensorE matmul with ablk (zero-padded to M=32) reduces over d: each
    matmul emits scores for 8 query rows, scattered at PSUM partition
    bases {0,32,64,96}.
  - VectorE copies exit PSUM strips to SBUF (fp16); 16 small TensorE
    "gather" matmuls with two constant 0/1 permutation matrices densify
    the scattered rows into one dense [128, 512] PSUM tile per pair.
  - ScalarE Exp with accum_out gives exp + row sums; VectorE reciprocal +
    tensor_scalar_mul normalizes; DMA out. All Exp ops are ordered after
    the Silu stream via nosync dep edges so the ACT table set switches
    only twice per kernel.

mask is all-False for this problem (spec fill=zeros): if a nonzero mask
is ever passed, an exact host-side renormalization fallback is applied.
scale is unused by the module.
"""

import os
import numpy as np
from contextlib import ExitStack

import concourse.bass as bass
import concourse.bacc as bacc
import concourse.mybir as mybir
import concourse.tile as tile
import bass_rust as _bass_rust
from concourse.bass_utils import run_bass_kernel_spmd

B, H, N, D = 16, 8, 256, 32
NCORES = 8
PAIRS = (B * H) // NCORES      # 16 (b,h) pairs per core
G = N // 4                     # 64 groups of 4 query rows
CHUNK_G = int(os.environ.get("GAT_CHUNK_G", "32"))   # groups per silu batch
NCHUNKS = G // CHUNK_G
NBANKS = 8                     # PSUM banks used per pair
ACT_COPY_EIGHTHS = int(os.environ.get("GAT_ACT_COPY_EIGHTHS", "0"))
SADD_BUFS = int(os.environ.get("GAT_SADD_BUFS", "4"))
SSIL_BUFS = int(os.environ.get("GAT_SSIL_BUFS", "4"))
EPOOL_BUFS = int(os.environ.get("GAT_EPOOL_BUFS", "3"))
PSTRIP_BUFS = int(os.environ.get("GAT_PSTRIP_BUFS", "3"))
DEFER = int(os.environ.get("GAT_DEFER", "16"))
ACT_DMA_PAIRS = int(os.environ.get("GAT_ACT_DMA_PAIRS", "0"))
FUSED_G = int(os.environ.get("GAT_FUSED_G", "8"))        # pairs per softmax block

FP16 = mybir.dt.float16
FP32 = mybir.dt.float32

_cache = {}


def build_program() -> bacc.Bacc:
    if "nc" in _cache:
        return _cache["nc"]
    nc = bacc.Bacc("TRN2")
    kT4_d = nc.declare_dram_parameter("kT4", [PAIRS, 128, N], FP16, isOutput=False)
    qT4_d = nc.declare_dram_parameter("qT4", [PAIRS, 128, G], FP32, isOutput=False)
    ablk_d = nc.declare_dram_parameter("ablk", [PAIRS, 128, 32], FP16, isOutput=False)
    perm_d = nc.declare_dram_parameter("perm", [128, 64], FP16, isOutput=False)
    out_d = nc.declare_dram_parameter("out", [PAIRS, N, N], FP32, isOutput=True)

    with ExitStack() as ctx:
        tc = ctx.enter_context(tile.TileContext(nc))
        inp = ctx.enter_context(tc.tile_pool(name="inp", bufs=3))
        cpool = ctx.enter_context(tc.tile_pool(name="cpool", bufs=1))
        sadd = ctx.enter_context(tc.tile_pool(name="sadd", bufs=SADD_BUFS))
        ssil = ctx.enter_context(tc.tile_pool(name="ssil", bufs=SSIL_BUFS))
        # strip psum tiles: 2 banks each, 3 in flight = 6 banks
        pstrip = ctx.enter_context(tc.tile_pool(name="pstrip", bufs=PSTRIP_BUFS, space="PSUM"))
        # dense psum tile: 1 bank, 2 in flight
        pdense = ctx.enter_context(tc.tile_pool(name="pdense", bufs=2, space="PSUM"))
        epool = ctx.enter_context(tc.tile_pool(name="epool", bufs=EPOOL_BUFS))
        dpool = ctx.enter_context(tc.tile_pool(name="dpool", bufs=DEFER + 2))
        xpool = ctx.enter_context(tc.tile_pool(name="xpool", bufs=3))
        rpool = ctx.enter_context(tc.tile_pool(name="rpool", bufs=3))
        spool = ctx.enter_context(tc.tile_pool(name="spool", bufs=DEFER + 2))

        exit_ctr = 0
        silu_insts = []
        # resident constants: permutation matrices for the gather matmuls
        pm = cpool.tile([128, 64], FP16, name="pm", tag="pm")
        nc.sync.dma_start(pm[:], perm_d[:])

        def phase1(p):
            kt = inp.tile([128, N], FP16, tag="kt")
            nc.sync.dma_start(kt[:], kT4_d[p])
            qt = inp.tile([128, G], FP32, tag="qt")
            nc.sync.dma_start(qt[:], qT4_d[p])
            ab = inp.tile([128, 32], FP16, tag="ab")
            nc.sync.dma_start(ab[:], ablk_d[p])

            Dn = dpool.tile([128, 2 * N], FP32, tag="dense")
            P2 = pdense.tile([128, 512], FP32, name="p2", tag="p2")
            pbanks = {}
            gq = 0  # running strip counter t
            nonlocal exit_ctr
            if p == 0:
                # fine-grained opening chunks: ScalarE starts sooner
                plan = [4, 4, 8]
            else:
                plan = []
            rem = G - sum(plan)
            while rem > 0:
                take = min(CHUNK_G, rem)
                plan.append(take)
                rem -= take
            assert sum(plan) == G
            g0 = 0
            first_chunk = True
            for csz in plan:
                Ss = ssil.tile([128, CHUNK_G * N], FP16, tag="ss")
                if p == 0 and g0 < FUSED_G:
                    # ramp: fused add+silu on ScalarE (bias = q column) so ACT
                    # starts right after the input DMA, no DVE dependency
                    for gl in range(csz):
                        g = g0 + gl
                        si = nc.scalar.activation(
                            Ss[:, gl * N:(gl + 1) * N], kt[:],
                            mybir.ActivationFunctionType.Silu,
                            bias=qt[:, g:g + 1],
                        )
                        silu_insts.append(si)
                else:
                    Sa = sadd.tile([128, CHUNK_G * N], FP16, tag="sa")
                    for gl in range(csz):
                        g = g0 + gl
                        nc.vector.tensor_scalar_add(
                            Sa[:, gl * N:(gl + 1) * N], kt[:], qt[:, g:g + 1]
                        )
                    si = nc.scalar.activation(
                        Ss[:, :csz * N], Sa[:, :csz * N],
                        mybir.ActivationFunctionType.Silu
                    )
                    silu_insts.append(si)
                first_chunk = False
                g0 += csz
                # strip matmuls: strip t covers query rows i = 8t + 4*g2 + m
                # at psum tile pi = t>>3, partition 32*(t&3)+m,
                # free 512*((t>>2)&1) + 256*g2 + j
                for tl in range(csz // 2):
                    t = gq
                    gq += 1
                    pi, sg, c_ = t >> 3, (t >> 2) & 1, t & 3
                    if (t & 7) == 0:
                        pbanks[pi] = pstrip.tile(
                            [128, 1024], FP32, name="pbank", tag="pbank"
                        )
                    # M=32 with zero-padded lhsT: rows 4..31 of each
                    # 32-block are written as zeros (keeps PSUM NaN-free
                    # for the gather matmuls)
                    nc.tensor.matmul(
                        pbanks[pi][32 * c_:32 * c_ + 32, 512 * sg:512 * sg + 512],
                        ab[:, :],
                        Ss[:, tl * 512:(tl + 1) * 512],
                        start=True, stop=True,
                        tile_position=(0, 32 * c_),
                    )
                    if (t & 7) == 7:
                        # tile complete: exit PSUM -> SBUF
                        Eb = epool.tile([128, 1024], FP16, tag="eb")
                        exit_ctr += 1
                        if (exit_ctr % 8) < ACT_COPY_EIGHTHS:
                            nc.scalar.copy(Eb[:], pbanks[pi][:, :])
                        else:
                            nc.vector.tensor_copy(Eb[:], pbanks[pi][:, :])
                        # gather matmuls: densify 64 rows of this tile into P2
                        # kappa = 2*sigma + g2 selects a 256-col block of Eb;
                        # target rows 64*(pi&1)+32*sigma + (8c+4*g2+m)
                        for kap in range(4):
                            sg2, g2 = kap >> 1, kap & 1
                            row0 = 64 * (pi & 1) + 32 * sg2
                            nc.tensor.matmul(
                                P2[row0:row0 + 32,
                                   256 * (pi >> 1):256 * (pi >> 1) + 256],
                                pm[:, 32 * g2:32 * g2 + 32],
                                Eb[:, 256 * kap:256 * kap + 256],
                                start=(g2 == 0), stop=(g2 == 1),
                                tile_position=(0, row0),
                                skip_group_check=True,
                            )
            # dense exit: P2 -> Dn (bitcast fp16 for DVE 2x byte copy)
            nc.vector.tensor_copy(Dn[:].bitcast(FP16), P2[:, :].bitcast(FP16))
            return Dn

        def phase2(p, Dn):
            X = xpool.tile([128, 2 * N], FP32, tag="x")
            sm = spool.tile([128, 4], FP32, tag="sm")
            for h2 in range(2):
                ei = nc.scalar.activation(
                    X[:, h2 * N:(h2 + 1) * N],
                    Dn[:, h2 * N:(h2 + 1) * N],
                    mybir.ActivationFunctionType.Exp,
                    accum_out=sm[:, h2:h2 + 1],
                )
                if exp_gate is not None:
                    # ordering-only edge: batch Exp ops after a chosen Silu
                    # in the static ACT stream (few table switches)
                    _bass_rust.add_dep_helper(
                        ei.ins, exp_gate.ins, sync=False,
                        reason="batch exp after silu (act table)",
                    )
            nc.vector.reciprocal(sm[:, 2:4], sm[:, 0:2])
            R = rpool.tile([128, 2 * N], FP32, tag="r")
            for h2 in range(2):
                nc.vector.tensor_scalar_mul(
                    R[:, h2 * N:(h2 + 1) * N],
                    X[:, h2 * N:(h2 + 1) * N],
                    sm[:, 2 + h2:3 + h2],
                )
            # late pairs issue their output DMA from the (by then idle)
            # ScalarE HWDGE queue to relieve the SP issue backlog at the tail
            eng = nc.scalar if p >= PAIRS - ACT_DMA_PAIRS else nc.sync
            for h2 in range(2):
                eng.dma_start(
                    out_d[p, 128 * h2:128 * (h2 + 1), :],
                    R[:, h2 * N:(h2 + 1) * N],
                )

        # phase1 for all pairs; exp batches released at two points to
        # shorten the end-of-kernel tail while keeping table switches rare
        dns = []
        split = int(os.environ.get("GAT_EXP_SPLIT", str(PAIRS - 4)))
        gate_idx = {}
        for p in range(PAIRS):
            dns.append((p, phase1(p)))
            gate_idx[p] = len(silu_insts) - 1
        exp_gate = None
        first_half = [i for i in range(PAIRS) if i < split]
        second_half = [i for i in range(PAIRS) if i >= split]
        if first_half:
            exp_gate = silu_insts[gate_idx[max(first_half)]]
            for p in first_half:
                phase2(p, dns[p][1])
        exp_gate = silu_insts[-1]
        for p in second_half:
            phase2(p, dns[p][1])

    nc.compile()
    _cache["nc"] = nc
    return nc


def prepare_in_maps(q, k, attention):
    q = np.asarray(q, dtype=np.float32)
    k = np.asarray(k, dtype=np.float32)
    a = np.asarray(attention, dtype=np.float32).reshape(H, D)
    BH = B * H
    qf = q.reshape(BH, N, D)
    kf = k.reshape(BH, N, D)
    # kT4[p, 32m+d, j] = k[p, j, d]
    kT4 = np.tile(kf.transpose(0, 2, 1), (1, 4, 1)).astype(np.float16)
    # qT4[p, 32m+d, g] = q[p, 4g+m, d]
    qT4 = (
        qf.reshape(BH, G, 4, D)
        .transpose(0, 2, 3, 1)
        .reshape(BH, 128, G)
        .astype(np.float32)
    )
    # ablk[p, 32m+d, m'] = (m==m') * a[h(p), d], zero-padded to 32 cols
    ab = np.zeros((BH, 128, 32), np.float16)
    hh = np.arange(BH) % H
    a16 = a.astype(np.float16)
    for m in range(4):
        ab[:, 32 * m:32 * (m + 1), m] = a16[hh]
    # permutation matrices for the gather matmuls:
    # perm[32c+m, 32*g2 + (8c+4g2+m)] = 1
    perm = np.zeros((128, 64), np.float16)
    for c in range(4):
        for g2 in range(2):
            for m in range(4):
                perm[32 * c + m, 32 * g2 + 8 * c + 4 * g2 + m] = 1.0
    in_maps = []
    for c in range(NCORES):
        s = slice(c * PAIRS, (c + 1) * PAIRS)
        in_maps.append(
            {
                "kT4": np.ascontiguousarray(kT4[s]),
                "qT4": np.ascontiguousarray(qT4[s]),
                "ablk": np.ascontiguousarray(ab[s]),
                "perm": perm,
            }
        )
    return in_maps


def unshard_output(results) -> np.ndarray:
    outs = [np.asarray(r["out"]) for r in results]
    return np.concatenate(outs, axis=0).reshape(B, H, N, N).astype(np.float32)


def kernel(q, k, scale, mask, attention) -> np.ndarray:
    nc = build_program()
    in_maps = prepare_in_maps(q, k, attention)
    res = run_bass_kernel_spmd(nc, in_maps, list(range(NCORES)))
    attn = unshard_output(res.results)
    mask = np.asarray(mask)
    if mask.any():
        # exact post-hoc masking: softmax with -inf masked scores equals
        # zeroing masked probabilities and renormalizing
        keep = ~np.broadcast_to(mask, attn.shape)
        kept = attn * keep
        denom = kept.sum(-1, keepdims=True)
        nkeep = keep.sum(-1, keepdims=True)
        uniform = np.where(nkeep > 0, keep / np.maximum(nkeep, 1), 1.0 / N)
        attn = np.where(denom > 0, kept / np.maximum(denom, 1e-38), uniform)
        attn = attn.astype(np.float32)
    return attn

